# revision 6
# baseline (speedup 1.0000x reference)
"""MultiHeadGraphAttention Trainium2 kernel.

Data-parallel over batch: core b computes batch element b (B=8, 8 cores).

Per-core math (one batch element, N=2048 nodes, U=256 units, H=8 heads, d=32):
  Q = x Wq, K = x Wk, V = x Wv                      (projections)
  sT[k,q]  = sum_d KT[d,k] QT[d,q]                  (scores, transposed layout)
  e        = exp(sT/sqrt(d)) * adjT                 (masked exp)
  ctxT[d,q] = sum_k V[k,d] e[k,q]  ;  Z[q] = sum_k e[k,q]
  out      = (ctxT/Z).T @ Wo + bo

v3 design (ACT-exp is the bottleneck engine; everything else hides under it):
  - scores for the 4 heads of a group are split into two half-tiles (heads
    j=0,1 -> sps_a banks 0-1, heads j=2,3 -> sps_b banks 2-3), each head's
    512-wide output owning a full PSUM bank (concurrent row-banded matmuls
    must not share a bank). Two FD=1024 ACTIVATEs per half; the spool pool's
    bufs=2 rotation lets the next iteration's scores_a overlap ACT_b, so the
    Scalar engine runs back-to-back with no PSUM-recycle bubble.
  - Z rows are produced PRE-BROADCAST: the ones stationary is [128,32] so the
    Z matmul writes Z replicated across each head's 32 partitions (same cost,
    cost is free-size-proportional; outputs stay partition-disjoint across
    the col-tiled j's). Normalization then needs no DMA broadcast.
  - adjacency mask arrives from host as bf16 (values 0/1 exact), so mask DMA
    is 8MB not 16MB and there is no on-device int32->bf16 conversion pass.
  - output projection per 128-row block with immediate DMA (no output tail).
PSUM: spool 2x2 banks + cps 2x1 + zps 2x1 = 8 banks.
"""

import sys

for p in ("/opt/trn_rl_repo",):
    if p not in sys.path:
        sys.path.insert(0, p)

from contextlib import ExitStack

import numpy as np
import ml_dtypes

import concourse.bass as bass
import concourse.mybir as mybir
import concourse.tile as tile
from concourse import bacc
from concourse.bass_utils import run_bass_kernel_spmd

B, N, U, H, D = 8, 2048, 256, 8, 32
NB = N // 128          # 16 key blocks of 128
QC = 4                 # q chunks
QW = N // QC           # 512 q per chunk
SCALE = 1.0 / np.sqrt(np.float32(D))

f32 = mybir.dt.float32
bf16 = mybir.dt.bfloat16
EXP = mybir.ActivationFunctionType.Exp
MULT = mybir.AluOpType.mult


def build_program():
    nc = bacc.Bacc("TRN2", target_bir_lowering=False, debug=False,
                   enable_asserts=False, num_devices=B)

    xT_d = nc.dram_tensor("xT", [U, N], f32, kind="ExternalInput").ap()
    mT_d = nc.dram_tensor("mT", [N, N], bf16, kind="ExternalInput").ap()
    wq_d = nc.dram_tensor("Wq", [U, U], f32, kind="ExternalInput").ap()
    wk_d = nc.dram_tensor("Wk", [U, U], f32, kind="ExternalInput").ap()
    wv_d = nc.dram_tensor("Wv", [U, U], f32, kind="ExternalInput").ap()
    wo_d = nc.dram_tensor("Wo", [U, U], f32, kind="ExternalInput").ap()
    bo_d = nc.dram_tensor("bo", [U], f32, kind="ExternalInput").ap()
    out_d = nc.dram_tensor("out", [N, U], f32, kind="ExternalOutput").ap()

    with tile.TileContext(nc) as tc:
        with ExitStack() as ctx:
            kernel_body(ctx, tc, xT_d, mT_d, wq_d, wk_d, wv_d, wo_d,
                        bo_d, out_d)
    nc.compile()
    return nc


def kernel_body(ctx, tc, xT_d, mT_d, wq_d, wk_d, wv_d, wo_d, bo_d, out_d):
    nc = tc.nc
    persist = ctx.enter_context(tc.tile_pool(name="persist", bufs=1))
    stage = ctx.enter_context(tc.tile_pool(name="stage", bufs=4))
    epool = ctx.enter_context(tc.tile_pool(name="epool", bufs=4))
    npool = ctx.enter_context(tc.tile_pool(name="npool", bufs=4))
    spool = ctx.enter_context(tc.tile_pool(name="spool", bufs=2, space="PSUM"))
    cpool = ctx.enter_context(tc.tile_pool(name="cpool", bufs=2, space="PSUM"))
    zpool = ctx.enter_context(tc.tile_pool(name="zpool", bufs=2, space="PSUM"))

    # ---- persistent SBUF tensors -------------------------------------------
    # projections, transposed: chunk g holds heads 4g..4g+3 (head 4g+j at
    # partitions 32j..32j+32)
    qT = [persist.tile([128, N], bf16, tag=f"qT{c}", name=f"qT{c}") for c in range(2)]
    kT = [persist.tile([128, N], bf16, tag=f"kT{c}", name=f"kT{c}") for c in range(2)]
    # V natural, bf16: head h of key block kb at cols kb*U + h*D
    v_sb = persist.tile([128, NB * U], bf16, tag="v")
    # bf16 adjacency mask, kb chunk at cols kb*N
    m_sb = persist.tile([128, NB * N], bf16, tag="m")
    # weights: [128, 2*256], feature chunk c at cols c*U
    w_sb = {}
    for nm, dram in (("wq", wq_d), ("wk", wk_d), ("wv", wv_d), ("wo", wo_d)):
        w_sb[nm] = persist.tile([128, 2 * U], f32, tag=nm, name=nm)
        for c in range(2):
            nc.sync.dma_start(w_sb[nm][:, c * U:(c + 1) * U],
                              dram[c * 128:(c + 1) * 128, :])
    bo_sb = persist.tile([1, U], f32, tag="bo")
    nc.sync.dma_start(bo_sb[:], bo_d.rearrange("(o n) -> o n", o=1))
    ones_bf = persist.tile([128, 32], bf16, tag="ones_bf")
    nc.vector.memset(ones_bf[:], 1.0)
    ones_f = persist.tile([1, 128], f32, tag="ones_f")
    nc.vector.memset(ones_f[:], 1.0)
    ctxn = [persist.tile([128, N], f32, tag=f"ctxn{c}", name=f"ctxn{c}")
            for c in range(2)]

    # ---- input staging ------------------------------------------------------
    xT = [stage.tile([128, N], f32, tag="stage", name=f"xT{c}") for c in range(2)]
    for c in range(2):
        nc.sync.dma_start(xT[c][:], xT_d[c * 128:(c + 1) * 128, :])
    # mask DMAs (8MB total) stream while projections run
    for kb in range(NB):
        nc.sync.dma_start(m_sb[:, kb * N:(kb + 1) * N],
                          mT_d[kb * 128:(kb + 1) * 128, :])

    # ---- projections --------------------------------------------------------
    for w, dst in (("wq", qT), ("wk", kT)):
        for mo in range(2):           # output chunk (128 rows of QT/KT)
            for half in range(2):     # 1024-wide halves
                ps = spool.tile([128, 1024], f32, tag="s")
                for nn in range(2):   # 512-wide slices (full bank each)
                    q0 = half * 1024 + nn * 512
                    for kc in range(2):
                        nc.tensor.matmul(
                            ps[:, nn * 512:(nn + 1) * 512],
                            w_sb[w][:, (kc * 2 + mo) * 128:(kc * 2 + mo + 1) * 128],
                            xT[kc][:, q0:q0 + 512],
                            start=(kc == 0), stop=(kc == 1))
                nc.scalar.copy(dst[mo][:, half * 1024:(half + 1) * 1024], ps[:])
    for kb in range(NB):              # V = x @ Wv, natural layout, bf16
        ps = cpool.tile([128, 512], f32, tag="c")
        for kc in range(2):
            nc.tensor.matmul(
                ps[:, :U],
                xT[kc][:, kb * 128:(kb + 1) * 128],
                w_sb["wv"][:, kc * U:(kc + 1) * U],
                start=(kc == 0), stop=(kc == 1))
        nc.vector.tensor_copy(v_sb[:, kb * U:(kb + 1) * U], ps[:, :U])

    # ---- main attention loop ------------------------------------------------
    for qc in range(QC):
        qs = qc * QW
        cps = [cpool.tile([128, QW], f32, tag="c", name=f"cps{g}_{qc}")
               for g in range(2)]
        zps = [zpool.tile([128, QW], f32, tag="z", name=f"zps{g}_{qc}")
               for g in range(2)]
        for kb in range(NB):
            for g in range(2):        # head groups: g=0 -> h0-3, g=1 -> h4-7
                for h2 in range(2):   # head pairs: j = 2*h2, 2*h2+1
                    sps = spool.tile([128, 1024], f32, tag="s")
                    for dj in range(2):
                        j = 2 * h2 + dj
                        nc.tensor.matmul(
                            sps[:, dj * 512:(dj + 1) * 512],
                            kT[g][32 * j:32 * (j + 1), kb * 128:(kb + 1) * 128],
                            qT[g][32 * j:32 * (j + 1), qs:qs + QW],
                            start=True, stop=True,
                            tile_position=(32 * j, 0))
                    e = epool.tile([128, 1024], bf16, tag="e")
                    nc.scalar.activation(e[:], sps[:], EXP, scale=float(SCALE))
                    me = m_sb[:, kb * N + qs:kb * N + qs + QW]
                    nc.vector.tensor_tensor(
                        e.rearrange("p (j q) -> p j q", j=2),
                        e.rearrange("p (j q) -> p j q", j=2),
                        me.unsqueeze(1).broadcast_to([128, 2, QW]), MULT)
                    for dj in range(2):
                        j = 2 * h2 + dj
                        ej = e[:, dj * 512:(dj + 1) * 512]
                        nc.tensor.matmul(
                            cps[g][32 * j:32 * (j + 1), :],
                            v_sb[:, kb * U + (4 * g + j) * D:
                                 kb * U + (4 * g + j + 1) * D],
                            ej, start=(kb == 0), stop=(kb == NB - 1),
                            tile_position=(0, 32 * j))
                        nc.tensor.matmul(
                            zps[g][32 * j:32 * (j + 1), :],
                            ones_bf[:], ej,
                            start=(kb == 0), stop=(kb == NB - 1),
                            tile_position=(0, 32 * j))
        # normalize: Z is pre-broadcast across each head's 32 partitions
        for g in range(2):
            zrec = npool.tile([128, QW], f32, tag="n", name=f"zrec{g}_{qc}")
            nc.vector.reciprocal_approx_fast(zrec[:], zps[g][:])
            nc.vector.tensor_tensor(ctxn[g][:, qs:qs + QW], cps[g][:],
                                    zrec[:], MULT)
        # out projection + store, per 128-row block of q
        for qb2 in range(4):
            qb = qc * 4 + qb2
            ops = zpool.tile([128, QW], f32, tag="z", name=f"ops{qc}_{qb2}")
            for c in range(2):
                nc.tensor.matmul(
                    ops[:, :U],
                    ctxn[c][:, qb * 128:(qb + 1) * 128],
                    w_sb["wo"][:, c * U:(c + 1) * U],
                    start=(c == 0), stop=False)
            nc.tensor.matmul(ops[:, :U], ones_f[:], bo_sb[:],
                             start=False, stop=True, skip_group_check=True)
            ob = stage.tile([128, U], f32, tag="ostage", name=f"ob{qc}_{qb2}")
            nc.vector.tensor_copy(ob[:], ops[:, :U])
            nc.sync.dma_start(out_d[qb * 128:(qb + 1) * 128, :], ob[:])


_CACHED = None


def _get_program():
    global _CACHED
    if _CACHED is None:
        _CACHED = build_program()
    return _CACHED


def kernel(node_features, adjacency_matrix, Wq, Wk, Wv, Wo, bo, **run_kwargs):
    nc = _get_program()
    xT = np.ascontiguousarray(np.transpose(node_features, (0, 2, 1)),
                              dtype=np.float32)
    adjT = np.transpose(adjacency_matrix, (0, 2, 1))
    mT = np.ascontiguousarray(adjT).astype(ml_dtypes.bfloat16)
    in_maps = []
    for b in range(B):
        in_maps.append({
            "xT": xT[b], "mT": mT[b],
            "Wq": np.asarray(Wq, np.float32), "Wk": np.asarray(Wk, np.float32),
            "Wv": np.asarray(Wv, np.float32), "Wo": np.asarray(Wo, np.float32),
            "bo": np.asarray(bo, np.float32),
        })
    res = run_bass_kernel_spmd(nc, in_maps, core_ids=list(range(B)), **run_kwargs)
    out = np.stack([res.results[b]["out"] for b in range(B)], axis=0)
    kernel.last_results = res
    return out
